# revision 14
# baseline (speedup 1.0000x reference)
"""BiLSTM-CRF loss on 8 Trainium2 NeuronCores (pure data parallel over batch).

Strategy (per core, batch shard B=64):
  Phase 0: embedding gather (indirect DMA row gather) + PE transpose -> xT [65, B*L]
           (row 64 = ones, so the LSTM bias rides the x-matmul).
  Loop 1:  fwd and bwd LSTM run together, partition-stacked: fwd batch rows on
           partitions 0..63, bwd batch rows on 64..127.  All four gates use one
           sigmoid (tanh(x) = 2*sigmoid(2x)-1 folded into the weights; hidden
           state is tracked as h' = h/2 so h' = (sigmoid(2c)-0.5)*sigmoid(z_o)).
           Per-step transposed hidden states stored (bf16) for the logits pass.
  Loop 2:  logits^T chunks [18, 512] = Wd'^T hT (+ -1e4 * invalid-mask via a
           K=1 matmul) -> exp(logits+bd) -> g;  ghat = g * onehot(labels).
  Loop 3:  CRF forward recurrence + gold-path score as two parallel scaled
           exp-domain chains: state [19, 128] (19th row = "graveyard" that
           captures the terminal mass when a sequence's mask ends), one 19x19
           constant-stationary matmul + one DVE multiply per step, periodic
           per-column rescaling with log-scale accumulation.
  ll = path_score - log_norm, gathered to the host.
"""

import numpy as np
import ml_dtypes
from contextlib import ExitStack

import concourse.bacc as bacc
import concourse.bass as bass
import concourse.tile as tile
from concourse import mybir
from concourse.bass_utils import run_bass_kernel_spmd
from concourse.masks import make_identity

AF = mybir.ActivationFunctionType
ALU = mybir.AluOpType
F32 = mybir.dt.float32
BF16 = mybir.dt.bfloat16
I32 = mybir.dt.int32

EMB = 64
RNN = 128
K = 18
NCORES = 8
B = 64          # batch rows per core
L_FULL = 256
V_FULL = 30001
G = 4 * RNN     # 512 gate columns per direction


def build_program(L=L_FULL, V=V_FULL, w_rescale=12, dbg=False):
    NT = B * L
    nc = bacc.Bacc("TRN2", target_bir_lowering=False, debug=False)
    dbg_t = {}
    if dbg:
        dbg_t["dxT"] = nc.dram_tensor("dxT", [EMB + 1, NT], BF16, kind="ExternalOutput")
        dbg_t["dhj"] = nc.dram_tensor("dhj", [128, L * 128], BF16, kind="ExternalOutput")
        dbg_t["dgg"] = nc.dram_tensor("dgg", [K + 1, 2 * NT], BF16, kind="ExternalOutput")
        dbg_t["dast"] = nc.dram_tensor("dast", [K + 1, 2 * B], F32, kind="ExternalOutput")
        dbg_t["dls"] = nc.dram_tensor("dls", [1, 2 * B], F32, kind="ExternalOutput")
        dbg_t["dr1"] = nc.dram_tensor("dr1", [1, 2 * B], F32, kind="ExternalOutput")
        dbg_t["dz1"] = nc.dram_tensor("dz1", [1, 2 * B], F32, kind="ExternalOutput")
        dbg_t["da13"] = nc.dram_tensor("da13", [K + 1, 2 * B], F32, kind="ExternalOutput")

    emb = nc.dram_tensor("emb", [V, EMB], F32, kind="ExternalInput")
    xidx = nc.dram_tensor("xidx", [128, NT // 128], I32, kind="ExternalInput")
    wx = nc.dram_tensor("wx", [EMB + 1, 2 * G], BF16, kind="ExternalInput")
    wh = nc.dram_tensor("wh", [RNN, 2 * G], F32, kind="ExternalInput")
    wd = nc.dram_tensor("wd", [2 * RNN, K], BF16, kind="ExternalInput")
    bdt = nc.dram_tensor("bdt", [K, 1], F32, kind="ExternalInput")
    tmat = nc.dram_tensor("tmat", [K, K], F32, kind="ExternalInput")
    ohd = nc.dram_tensor("ohd", [K, NT], BF16, kind="ExternalInput")
    pend = nc.dram_tensor("pend", [1, NT], BF16, kind="ExternalInput")
    grvd = nc.dram_tensor("grvd", [1, NT], BF16, kind="ExternalInput")
    llo = nc.dram_tensor("ll", [1, B], F32, kind="ExternalOutput")

    with tile.TileContext(nc) as tc, ExitStack() as ctx:
        const = ctx.enter_context(tc.tile_pool(name="const", bufs=1))
        persist = ctx.enter_context(tc.tile_pool(name="persist", bufs=1))

        ident = const.tile([128, 128], F32)
        make_identity(nc, ident[:])
        wx_sb = const.tile([EMB + 1, 2 * G], BF16)
        nc.sync.dma_start(wx_sb[:], wx[:])
        wh_sb = const.tile([RNN, 2 * G], F32)
        nc.sync.dma_start(wh_sb[:], wh[:])
        wdf_sb = const.tile([RNN, K], BF16)
        nc.sync.dma_start(wdf_sb[:], wd[0:RNN, :])
        wdb_sb = const.tile([RNN, K], BF16)
        nc.sync.dma_start(wdb_sb[:], wd[RNN:2 * RNN, :])
        bdt_sb = const.tile([K, 1], F32)
        nc.sync.dma_start(bdt_sb[:], bdt[:])
        ones18 = const.tile([1, K], BF16)
        nc.vector.memset(ones18[:], 1.0)
        ones19 = const.tile([1, K + 1], F32)
        nc.vector.memset(ones19[:], 1.0)

        # Maug [19,33]: cols 0..17 = exp(T), col 18 = grave (ones: live-sum +
        # grave passthrough), col 32 = colsum readout (ones) placed at
        # partition 32 of the matmul output so PSUM row reads are aligned.
        MS = 33
        tm_sb = const.tile([K, K], F32)
        nc.sync.dma_start(tm_sb[:], tmat[:])
        maug = const.tile([K + 1, MS], F32)
        nc.vector.memset(maug[:], 0.0)
        nc.scalar.activation(maug[0:K, 0:K], tm_sb[:], AF.Exp)
        nc.vector.memset(maug[:, K:K + 1], 1.0)
        nc.vector.memset(maug[:, MS - 1:MS], 1.0)

        # persistent store: col t*128+b -> fwd hT for b<64, bwd hT for 64+b
        hjoint = persist.tile([128, L * 128], BF16)

        # ---------------- Phase 0 + Loop 1 (xT lives only here) -------------
        with tc.tile_pool(name="xT", bufs=1) as xpool:
            xT = xpool.tile([EMB + 1, NT], BF16)
            nc.vector.memset(xT[EMB:EMB + 1, :], 1.0)
            with (
                tc.tile_pool(name="gath", bufs=4) as gpool0,
                tc.tile_pool(name="gps", bufs=4, space="PSUM") as gps0,
                tc.tile_pool(name="idx", bufs=1) as ipool,
            ):
                idxt = ipool.tile([128, NT // 128], I32)
                nc.sync.dma_start(idxt[:], xidx[:])
                for k in range(NT // 128):
                    xg = gpool0.tile([128, EMB], F32, tag="xg")
                    nc.gpsimd.indirect_dma_start(
                        out=xg[:],
                        out_offset=None,
                        in_=emb[:],
                        in_offset=bass.IndirectOffsetOnAxis(ap=idxt[:, k:k + 1], axis=0),
                    )
                    xps = gps0.tile([EMB, 128], F32, tag="xps")
                    nc.tensor.transpose(xps[:], xg[:], ident[:])
                    nc.vector.tensor_copy(xT[0:EMB, k * 128:(k + 1) * 128], xps[:])

            if dbg:
                nc.sync.dma_start(dbg_t["dxT"][:], xT[:])
            with (
                tc.tile_pool(name="state", bufs=2) as spool,
                tc.tile_pool(name="zps", bufs=2, space="PSUM") as zpool,
                tc.tile_pool(name="tps", bufs=2, space="PSUM") as tpool,
                tc.tile_pool(name="gates", bufs=3) as gpool,
            ):
                h_cur = spool.tile([128, 128], F32, tag="h")
                nc.vector.memset(h_cur[:], 0.0)
                c_cur = spool.tile([128, RNN], F32, tag="c")
                nc.vector.memset(c_cur[:], 0.0)
                for i in range(L):
                    tf, tb = i, L - 1 - i
                    z = zpool.tile([128, G], F32)
                    nc.tensor.matmul(z[0:B, :], lhsT=xT[:, tf * B:(tf + 1) * B],
                                     rhs=wx_sb[:, 0:G], start=True, stop=False)
                    nc.tensor.matmul(z[0:B, :], lhsT=h_cur[:, 0:B],
                                     rhs=wh_sb[:, 0:G], start=False, stop=True)
                    nc.tensor.matmul(z[B:2 * B, :], lhsT=xT[:, tb * B:(tb + 1) * B],
                                     rhs=wx_sb[:, G:2 * G], start=True, stop=False,
                                     tile_position=(0, B))
                    nc.tensor.matmul(z[B:2 * B, :], lhsT=h_cur[:, B:2 * B],
                                     rhs=wh_sb[:, G:2 * G], start=False, stop=True,
                                     tile_position=(0, B))
                    s = gpool.tile([128, G], F32, tag="s")
                    nc.scalar.activation(s[:], z[:], AF.Sigmoid)
                    v = gpool.tile([128, RNN], F32, tag="v")
                    nc.vector.scalar_tensor_tensor(
                        v[:], in0=s[:, 3 * RNN:4 * RNN], scalar=-0.5,
                        in1=s[:, 0:RNN], op0=ALU.add, op1=ALU.mult)
                    p_ = gpool.tile([128, RNN], F32, tag="p")
                    nc.vector.tensor_tensor(p_[:], s[:, RNN:2 * RNN], c_cur[:],
                                            op=ALU.mult)
                    c_new = spool.tile([128, RNN], F32, tag="c")
                    nc.vector.scalar_tensor_tensor(
                        c_new[:], in0=v[:], scalar=2.0, in1=p_[:],
                        op0=ALU.mult, op1=ALU.add)
                    sc = gpool.tile([128, RNN], F32, tag="sc")
                    nc.scalar.activation(sc[:], c_new[:], AF.Sigmoid, scale=2.0)
                    hp = gpool.tile([128, RNN], F32, tag="hp")
                    nc.vector.scalar_tensor_tensor(
                        hp[:], in0=sc[:], scalar=-0.5, in1=s[:, 2 * RNN:3 * RNN],
                        op0=ALU.add, op1=ALU.mult)
                    hps = tpool.tile([128, 128], F32)
                    nc.tensor.transpose(hps[:], hp[:], ident[:])
                    h_new = spool.tile([128, 128], F32, tag="h")
                    nc.vector.tensor_copy(h_new[:], hps[:])
                    nc.vector.tensor_copy(
                        hjoint[:, tf * 128:tf * 128 + B], h_new[:, 0:B])
                    nc.vector.tensor_copy(
                        hjoint[:, tb * 128 + B:tb * 128 + 2 * B], h_new[:, B:2 * B])
                    h_cur, c_cur = h_new, c_new

        if dbg:
            nc.sync.dma_start(dbg_t["dhj"][:], hjoint[:])
        # ---------------- Loop 2: logits -> g, ghat ------------------------
        TCH = 8
        ggpool = ctx.enter_context(tc.tile_pool(name="gg", bufs=1))
        gg = ggpool.tile([K + 1, 2 * NT], BF16)   # [alpha-g | path-ghat], t-major
        nc.sync.dma_start(gg[K:K + 1, 0:NT], grvd[:])
        nc.sync.dma_start(gg[K:K + 1, NT:2 * NT], grvd[:])
        with (
            tc.tile_pool(name="l2in", bufs=3) as l2pool,
            tc.tile_pool(name="lps", bufs=4, space="PSUM") as lpool,
        ):
            hj3 = hjoint[:].rearrange("p (t c) -> p t c", c=128)
            for q in range(L // TCH):
                t0 = q * TCH
                cw = TCH * B
                ohc = l2pool.tile([K, cw], BF16, tag="ohc")
                nc.sync.dma_start(ohc[:], ohd[:, t0 * B:t0 * B + cw])
                penc = l2pool.tile([1, cw], BF16, tag="penc")
                nc.sync.dma_start(penc[:], pend[:, t0 * B:t0 * B + cw])
                lp = lpool.tile([K, cw], F32)
                nc.tensor.matmul(lp[:], lhsT=wdf_sb[:],
                                 rhs=hj3[:, t0:t0 + TCH, 0:B],
                                 start=True, stop=False)
                nc.tensor.matmul(lp[:], lhsT=wdb_sb[:],
                                 rhs=hj3[:, t0:t0 + TCH, B:2 * B],
                                 start=False, stop=False)
                nc.tensor.matmul(lp[:], lhsT=ones18[:], rhs=penc[:],
                                 start=False, stop=True)
                nc.scalar.activation(gg[0:K, t0 * B:t0 * B + cw], lp[:],
                                     AF.Exp, bias=bdt_sb[:])
                nc.vector.tensor_tensor(
                    gg[0:K, NT + t0 * B:NT + t0 * B + cw],
                    gg[0:K, t0 * B:t0 * B + cw],
                    ohc[:], op=ALU.mult)

        if dbg:
            nc.sync.dma_start(dbg_t["dgg"][:], gg[:])
        # ---------------- Loop 3: CRF chains -------------------------------
        gg3 = gg[:].rearrange("p (c n) -> p c n", c=2)
        with (
            tc.tile_pool(name="crf", bufs=3) as cpool,
            tc.tile_pool(name="crfps", bufs=4, space="PSUM") as cps,
        ):
            ast = cpool.tile([K + 1, 2 * B], F32, tag="ast")
            nc.vector.memset(ast[:], 0.0)
            nc.vector.tensor_copy(ast[0:K, :].rearrange("p (c n) -> p c n", c=2),
                                  gg3[0:K, :, 0:B])
            ls = cpool.tile([1, 2 * B], F32, tag="ls")
            nc.vector.memset(ls[:], 0.0)
            for t in range(1, L):
                pa = cps.tile([MS, 2 * B], F32, tag="pa")
                nc.tensor.matmul(pa[:], lhsT=maug[:], rhs=ast[:],
                                 start=True, stop=True)
                pa3 = pa[0:K + 1, :].rearrange("p (c n) -> p c n", c=2)
                gsl = gg3[:, :, t * B:(t + 1) * B]
                a_new = cpool.tile([K + 1, 2 * B], F32, tag="ast")
                an3 = a_new[:].rearrange("p (c n) -> p c n", c=2)
                if t % w_rescale == 0:
                    zrow = cpool.tile([1, 2 * B], F32, tag="zrow")
                    nc.vector.tensor_copy(zrow[:], pa[MS - 1:MS, :])
                    r = cpool.tile([1, 2 * B], F32, tag="r")
                    nc.vector.reciprocal(r[:], zrow[:])
                    if dbg and t == w_rescale:
                        nc.sync.dma_start(dbg_t["dz1"][:], zrow[:])
                        nc.sync.dma_start(dbg_t["dr1"][:], r[:])
                    pr = cps.tile([K + 1, 2 * B], F32, tag="pr")
                    nc.tensor.matmul(pr[:], lhsT=ones19[:], rhs=r[:],
                                     start=True, stop=True)
                    lnr = cpool.tile([1, 2 * B], F32, tag="lnr")
                    nc.scalar.activation(lnr[:], r[:], AF.Ln)
                    ls_new = cpool.tile([1, 2 * B], F32, tag="ls")
                    nc.vector.tensor_tensor(ls_new[:], ls[:], lnr[:],
                                            op=ALU.subtract)
                    ls = ls_new
                    atmp = cpool.tile([K + 1, 2 * B], F32, tag="atmp")
                    nc.vector.tensor_tensor(
                        atmp[:].rearrange("p (c n) -> p c n", c=2), pa3, gsl,
                        op=ALU.mult)
                    nc.vector.tensor_tensor(a_new[:], atmp[:], pr[:],
                                            op=ALU.mult)
                else:
                    nc.vector.tensor_tensor(an3, pa3, gsl, op=ALU.mult)
                ast = a_new
                if dbg and t == w_rescale + 1:
                    nc.sync.dma_start(dbg_t["da13"][:], ast[:])
            pf = cps.tile([MS, 2 * B], F32, tag="pa")
            nc.tensor.matmul(pf[:], lhsT=maug[:], rhs=ast[:], start=True, stop=True)
            lnt = cpool.tile([1, 2 * B], F32, tag="lnt")
            nc.scalar.activation(lnt[:], pf[MS - 1:MS, :], AF.Ln)
            tot = cpool.tile([1, 2 * B], F32, tag="tot")
            nc.vector.tensor_tensor(tot[:], lnt[:], ls[:], op=ALU.add)
            ll_sb = cpool.tile([1, B], F32, tag="ll")
            nc.vector.tensor_tensor(ll_sb[:], tot[:, B:2 * B], tot[:, 0:B],
                                    op=ALU.subtract)
            nc.sync.dma_start(llo[:], ll_sb[:])
            if dbg:
                nc.sync.dma_start(dbg_t["dast"][:], ast[:])
                nc.sync.dma_start(dbg_t["dls"][:], ls[:])

    nc.compile()
    return nc


# ---------------------------------------------------------------------------
# host side
# ---------------------------------------------------------------------------

def _pack_dir(Wx, Wh, b):
    i, f, g, o = np.split(np.asarray(Wx, np.float32), 4, axis=1)
    wxp = np.concatenate([i, f, o, 2.0 * g], axis=1)
    i, f, g, o = np.split(np.asarray(Wh, np.float32), 4, axis=1)
    whp = np.concatenate([2.0 * i, 2.0 * f, 2.0 * o, 4.0 * g], axis=1)
    bi, bf_, bg, bo = np.split(np.asarray(b, np.float32), 4)
    bp = np.concatenate([bi, bf_, bo, 2.0 * bg])
    return np.concatenate([wxp, bp[None, :]], axis=0), whp


def make_in_maps(inputs, labels, E, Wx_f, Wh_f, b_f, Wx_b, Wh_b, b_b, Wd, bd, T,
                 L=L_FULL):
    NT = B * L
    bf16 = ml_dtypes.bfloat16
    wxf, whf = _pack_dir(Wx_f, Wh_f, b_f)
    wxb, whb = _pack_dir(Wx_b, Wh_b, b_b)
    wx = np.concatenate([wxf, wxb], axis=1).astype(bf16)
    wh = np.concatenate([whf, whb], axis=1).astype(np.float32)
    wd = (2.0 * np.asarray(Wd, np.float32)).astype(bf16)
    bdt = np.asarray(bd, np.float32).reshape(K, 1)
    tmat = np.asarray(T, np.float32)
    emb = np.ascontiguousarray(np.asarray(E, np.float32))
    tok = np.asarray(inputs).astype(np.int32)
    lab = np.asarray(labels).astype(np.int32)

    in_maps = []
    for c in range(NCORES):
        tk = tok[c * B:(c + 1) * B]          # [B, L]
        lb = lab[c * B:(c + 1) * B]
        ids = np.ascontiguousarray(tk.T).reshape(-1)          # t-major [NT]
        xidx = np.ascontiguousarray(ids.reshape(NT // 128, 128).T).astype(np.int32)
        labt = np.ascontiguousarray(lb.T).reshape(-1)
        oh = (labt[None, :] == np.arange(K, dtype=np.int64)[:, None])
        lens = (lb != 0).sum(axis=1)                          # [B]
        apf = (np.arange(L)[None, :] >= lens[:, None])        # [B, L] invalid
        apt = np.ascontiguousarray(apf.T).reshape(-1).astype(np.float32)
        pen = (-10000.0 * apt)[None, :]
        pen[0, 0:B] = 0.0
        pen = pen.astype(bf16)
        in_maps.append(dict(
            emb=emb, xidx=xidx, wx=wx, wh=wh, wd=wd, bdt=bdt, tmat=tmat,
            ohd=oh.astype(bf16), pend=pen, grvd=apt[None, :].astype(bf16),
        ))
    return in_maps


_PROG = None


def _get_prog():
    global _PROG
    if _PROG is None:
        _PROG = build_program()
    return _PROG


def kernel(inputs, labels, E, Wx_f, Wh_f, b_f, Wx_b, Wh_b, b_b, Wd, bd, T):
    nc = _get_prog()
    in_maps = make_in_maps(inputs, labels, E, Wx_f, Wh_f, b_f,
                           Wx_b, Wh_b, b_b, Wd, bd, T)
    res = run_bass_kernel_spmd(nc, in_maps, core_ids=list(range(NCORES)))
    ll = np.concatenate([res.results[c]["ll"].reshape(B) for c in range(NCORES)])
    return ll.astype(np.float32), np.asarray(T, np.float32)


# numpy mini-reference (float64) for testing at arbitrary L/V ----------------

def ref_numpy(inputs, labels, E, Wx_f, Wh_f, b_f, Wx_b, Wh_b, b_b, Wd, bd, T):
    f = np.float64
    tok = np.asarray(inputs); lab = np.asarray(labels)
    E = np.asarray(E, f); T = np.asarray(T, f)
    Bf, Lf = tok.shape

    def sig(x):
        return 1.0 / (1.0 + np.exp(-x))

    def lstm(x, Wx, Wh, b, reverse):
        Wx = np.asarray(Wx, f); Wh = np.asarray(Wh, f); b = np.asarray(b, f)
        h = np.zeros((Bf, RNN), f); c = np.zeros((Bf, RNN), f)
        hs = np.zeros((Lf, Bf, RNN), f)
        order = range(Lf - 1, -1, -1) if reverse else range(Lf)
        for t in order:
            z = x[t] @ Wx + h @ Wh + b
            i, fg, g, o = np.split(z, 4, axis=1)
            c = sig(fg) * c + sig(i) * np.tanh(g)
            h = sig(o) * np.tanh(c)
            hs[t] = h
        return hs

    x = E[tok].transpose(1, 0, 2)             # [L, B, E]
    hf = lstm(x, Wx_f, Wh_f, b_f, False)
    hb = lstm(x, Wx_b, Wh_b, b_b, True)
    h = np.concatenate([hf, hb], axis=2)      # [L, B, 2R]
    logits = h.transpose(1, 0, 2) @ np.asarray(Wd, f) + np.asarray(bd, f)
    lens = (lab != 0).sum(axis=1)
    pos = np.arange(Lf)[None, :] < lens[:, None]
    unary = np.take_along_axis(logits, lab[..., None], axis=2)[..., 0]
    unary = (unary * pos).sum(axis=1)
    binary = (T[lab[:, :-1], lab[:, 1:]] * pos[:, 1:]).sum(axis=1)
    alpha = logits[:, 0, :].copy()
    for t in range(1, Lf):
        new = np.log(np.exp(alpha[:, :, None] - alpha.max(1)[:, None, None]
                            + T[None]).sum(axis=1)) \
            + alpha.max(1)[:, None] + logits[:, t, :]
        alpha = np.where(pos[:, t][:, None], new, alpha)
    mx = alpha.max(1)
    log_norm = np.log(np.exp(alpha - mx[:, None]).sum(1)) + mx
    return unary + binary - log_norm
